# revision 4
# baseline (speedup 1.0000x reference)
"""ACT-R recurrence kernel, v2: transposed layout + PE-maintained diff state.

Layout: batch on partitions ([128 p] x [16 bc] x [128 j] free), per core
B=2048.  All TRN2 elementwise/reduce costs scale with free size only.

D state lives in PSUM [128, 16, 128] f32, maintained by the idle PE:
  step i: psum_D[:, :, 0:i] += gap_i  (identity-stationary matmul, bf16
  moving with a stride-0 broadcast AP; one accumulating matmul per bank).
After increment i, psum_D[:, :, j] = sp_i - sp_j for j < i.  Ln reads PSUM.

Per step (lag-1 split, S_i = P_i + F_i):
  Ln_i   (ACT):  L_i = Ln(scale * psum_D[0:i])            -> f16 ring
  QF_i   (DVE):  L_i[i-1] * NEG[i-1]          [128,16,1]
  ExpF_i (ACT):  F_i = Exp(QF_i)              [128,16,1]
  mul_i+1(DVE):  QB = L_{i+1}[0:i] * NEG[0:i]  f16 (2x mode)
  ExpB   (ACT):  TB = Exp(QB)                  f16
  red    (DVE):  P_{i+1} = reduce_add_X(TB)    f32
  stt_i  (DVE):  NEG[i] = -c*F_i + NEGP[i]     (fused)
  negp   (DVE):  NEGP[i+1] = -c*P_{i+1} - a
Final: S = (NEG + a)/(-c); out = Sigmoid((Ln(S) - tau)/s).
Ln and Exp share one activation table (get_activation_tables patch).
Single input: transposed bf16 gaps.  No gpsimd, no collectives.
"""

import sys

for _p in ("/opt/trn_rl_repo",):
    if _p not in sys.path:
        sys.path.insert(0, _p)

import numpy as np
from contextlib import ExitStack

import concourse.hw_specs as hw_specs
import concourse.bacc as bacc_mod
from concourse import mybir

_orig_gat = hw_specs.get_activation_tables


def _patched_gat(arch):
    tabs = _orig_gat(arch)
    out = {}
    ln_t = mybir.ActivationFunctionType.Ln
    exp_t = mybir.ActivationFunctionType.Exp
    for name, funcs in tabs.items():
        f = set(funcs)
        if name != "natural_log_exp_and_others":
            f.discard(ln_t)
            f.discard(exp_t)
        out[name] = f
    return out


bacc_mod.get_activation_tables = _patched_gat

import concourse.bass as bass
import concourse.bacc as bacc
import concourse.tile as tile
from concourse.bass_utils import run_bass_kernel_spmd

S = 128
B_FULL = 16384
N_CORES = 8
B = B_FULL // N_CORES  # 2048 per core
P = 128
NC = B // P  # 16

F32 = mybir.dt.float32
F16 = mybir.dt.float16
BF16 = mybir.dt.bfloat16
AF = mybir.ActivationFunctionType
ALU = mybir.AluOpType

NL = 4  # L ring slots (W=3 lookahead on the D/Ln pipeline)
W = 3


def build_kernel(a, c, s, tau, h, repeat=1):
    scale = 86400.0 * float(h)
    nc = bacc.Bacc()

    grt_in = nc.declare_dram_parameter("grt", [P, NC, S], BF16, isOutput=False)
    eye_in = nc.declare_dram_parameter("eye", [P, P], BF16, isOutput=False)
    out_ext = nc.declare_dram_parameter("out", [P, NC, S], F32, isOutput=True)

    with ExitStack() as ctx:
        tc = ctx.enter_context(tile.TileContext(nc))
        pool = ctx.enter_context(tc.tile_pool(name="p", bufs=1))

        GRT = pool.tile([P, NC, S], BF16)
        nc.sync.dma_start(out=GRT[:], in_=grt_in[:])
        EYE = pool.tile([P, P], BF16)
        nc.sync.dma_start(out=EYE[:], in_=eye_in[:])

        NEG = pool.tile([P, NC, S], F16)
        NEGP = pool.tile([P, NC, S], F32)

        LR = [pool.tile([P, NC, S], F16, name=f"LR{r}") for r in range(NL)]
        QB = [pool.tile([P, NC, S], F16, name=f"QB{r}") for r in range(2)]
        TB = [pool.tile([P, NC, S], F16, name=f"TB{r}") for r in range(2)]
        QF = [pool.tile([P, NC, 1], F16, name=f"QF{r}") for r in range(2)]
        TF = [pool.tile([P, NC, 1], F32, name=f"TF{r}") for r in range(2)]
        PR = [pool.tile([P, NC, 1], F32, name=f"PR{r}") for r in range(2)]
        BIAS = pool.tile([P, 1], F32)
        nc.vector.memset(BIAS[:], -float(tau) / float(s))

        psum_D = ctx.enter_context(nc.psum_tensor([P, NC, S], F32))

        def emit_inc(k):
            # psum_D[:, :, 0:k] += gap_k ; one matmul per bc chunk (each
            # [128, k] output stays inside a single 512-col psum bank).
            for bc in range(NC):
                outap = psum_D[:, bc, 0:k]
                src = GRT[:, bc, k : k + 1]
                mov = bass.AP(src.tensor, src.offset, [src.ap[0], [0, k]])
                nc.tensor.matmul(
                    outap, EYE[:], mov,
                    start=False, stop=False, skip_group_check=True,
                )

        def emit_ln(i, r):
            nc.scalar.activation(
                LR[r][:, :, 0:i], psum_D[:, :, 0:i], AF.Ln, scale=scale
            )

        for _rep in range(repeat):
            nc.vector.memset(NEG[:], -float(a))
            nc.vector.memset(NEGP[:], -float(a))
            nc.vector.memset(psum_D[:], 0.0)

            # prime the D/Ln pipeline W steps deep
            for k in range(1, min(W + 1, S)):
                emit_inc(k)
                emit_ln(k, k % NL)

            for i in range(1, S):
                r = i % NL
                x = i % 2
                # advance D for step i+W (waits Ln_{i+W-1}, which was
                # emitted last iteration — PE runs in ACT's shadow)
                if i + W < S:
                    emit_inc(i + W)
                # fresh chain for step i
                nc.vector.tensor_tensor(
                    out=QF[x][:], in0=LR[r][:, :, i - 1 : i],
                    in1=NEG[:, :, i - 1 : i], op=ALU.mult,
                )
                nc.scalar.activation(TF[x][:], QF[x][:], AF.Exp)
                # bulk for step i+1 over j < i
                if i + 1 < S:
                    r1 = (i + 1) % NL
                    x1 = (i + 1) % 2
                    nc.vector.tensor_tensor(
                        out=QB[x1][:, :, 0:i], in0=LR[r1][:, :, 0:i],
                        in1=NEG[:, :, 0:i], op=ALU.mult,
                    )
                    nc.scalar.activation(
                        TB[x1][:, :, 0:i], QB[x1][:, :, 0:i], AF.Exp
                    )
                if i + W < S:
                    emit_ln(i + W, (i + W) % NL)
                nc.vector.scalar_tensor_tensor(
                    out=NEG[:, :, i : i + 1], in0=TF[x][:], scalar=-float(c),
                    in1=NEGP[:, :, i : i + 1], op0=ALU.mult, op1=ALU.add,
                )
                if i + 1 < S:
                    nc.vector.tensor_reduce(
                        out=PR[x1][:], in_=TB[x1][:, :, 0:i],
                        axis=mybir.AxisListType.X, op=ALU.add,
                    )
                    nc.vector.tensor_scalar(
                        out=NEGP[:, :, i + 1 : i + 2], in0=PR[x1][:],
                        scalar1=-float(c), scalar2=-float(a),
                        op0=ALU.mult, op1=ALU.add,
                    )

        # epilogue
        SS = pool.tile([P, NC, S], F32)
        nc.vector.tensor_scalar(
            out=SS[:, :, 1:S], in0=NEG[:, :, 1:S],
            scalar1=float(a), scalar2=-1.0 / float(c),
            op0=ALU.add, op1=ALU.mult,
        )
        M = pool.tile([P, NC, S], F32)
        nc.scalar.activation(M[:, :, 1:S], SS[:, :, 1:S], AF.Ln)
        O = pool.tile([P, NC, S], F32)
        nc.vector.memset(O[:, :, 0:1], 0.0)
        nc.scalar.activation(
            O[:, :, 1:S], M[:, :, 1:S], AF.Sigmoid,
            scale=1.0 / float(s), bias=BIAS[:],
        )
        nc.sync.dma_start(out=out_ext[:], in_=O[:])

    nc.compile()
    return nc


def make_in_maps(sp: np.ndarray) -> list:
    bf = mybir.dt.np(BF16)
    eye = np.eye(P, dtype=np.float32).astype(bf)
    in_maps = []
    for ci in range(N_CORES):
        shard = sp[:, ci * B : (ci + 1) * B].astype(np.float32)  # [S, B]
        gaps = np.empty_like(shard)
        gaps[0] = shard[0]
        gaps[1:] = shard[1:] - shard[:-1]
        # grt[p, bc, j] = gaps[j, bc*128 + p]
        grt = np.ascontiguousarray(
            gaps.reshape(S, NC, P).transpose(2, 1, 0)
        ).astype(bf)
        in_maps.append({"grt": grt, "eye": eye})
    return in_maps


def kernel(sp: np.ndarray, w: np.ndarray) -> np.ndarray:
    sp = np.ascontiguousarray(sp, dtype=np.float32)
    w = np.asarray(w, dtype=np.float32)
    a, c, s, tau, h = (float(x) for x in w)

    nc = build_kernel(a, c, s, tau, h)
    in_maps = make_in_maps(sp)

    res = run_bass_kernel_spmd(nc, in_maps, core_ids=list(range(N_CORES)))
    outs = []
    for ci in range(N_CORES):
        o = res.results[ci]["out"]  # [P, NC, S]
        outs.append(o.transpose(2, 1, 0).reshape(S, B)[1:S])
    return np.concatenate(outs, axis=1).astype(np.float32)


if __name__ == "__main__":
    rng = np.random.default_rng(0)
    spt = np.cumsum(rng.uniform(0.1, 5.0, (S, B_FULL)).astype(np.float32), axis=0)
    wt = np.asarray(
        [0.176786766570677, 0.216967308403809, 0.254893976981164,
         -0.704205679427144, 0.025], dtype=np.float32)
    o = kernel(spt, wt)
    print(o.shape, o.dtype, o[:3, :3])


# revision 5
# speedup vs baseline: 1.0378x; 1.0378x over previous
"""ACT-R recurrence kernel, v2: transposed layout + PE-maintained diff state.

Layout: batch on partitions ([128 p] x [16 bc] x [128 j] free), per core
B=2048.  All TRN2 elementwise/reduce costs scale with free size only.

D state lives in PSUM [128, 16, 128] f32, maintained by the idle PE:
  step i: psum_D[:, :, 0:i] += gap_i  (identity-stationary matmul, bf16
  moving with a stride-0 broadcast AP; one accumulating matmul per bank).
After increment i, psum_D[:, :, j] = sp_i - sp_j for j < i.  Ln reads PSUM.

Per step (lag-1 split, S_i = P_i + F_i):
  Ln_i   (ACT):  L_i = Ln(scale * psum_D[0:i])            -> f16 ring
  QF_i   (DVE):  L_i[i-1] * NEG[i-1]          [128,16,1]
  ExpF_i (ACT):  F_i = Exp(QF_i)              [128,16,1]
  mul_i+1(DVE):  QB = L_{i+1}[0:i] * NEG[0:i]  f16 (2x mode)
  ExpB   (ACT):  TB = Exp(QB)                  f16
  red    (DVE):  P_{i+1} = reduce_add_X(TB)    f32
  stt_i  (DVE):  NEG[i] = -c*F_i + NEGP[i]     (fused)
  negp   (DVE):  NEGP[i+1] = -c*P_{i+1} - a
Final: S = (NEG + a)/(-c); out = Sigmoid((Ln(S) - tau)/s).
Ln and Exp share one activation table (get_activation_tables patch).
Single input: transposed bf16 gaps.  No gpsimd, no collectives.
"""

import sys

for _p in ("/opt/trn_rl_repo",):
    if _p not in sys.path:
        sys.path.insert(0, _p)

import numpy as np
from contextlib import ExitStack

import concourse.hw_specs as hw_specs
import concourse.bacc as bacc_mod
from concourse import mybir

_orig_gat = hw_specs.get_activation_tables


def _patched_gat(arch):
    tabs = _orig_gat(arch)
    out = {}
    ln_t = mybir.ActivationFunctionType.Ln
    exp_t = mybir.ActivationFunctionType.Exp
    for name, funcs in tabs.items():
        f = set(funcs)
        if name != "natural_log_exp_and_others":
            f.discard(ln_t)
            f.discard(exp_t)
        out[name] = f
    return out


bacc_mod.get_activation_tables = _patched_gat

import concourse.bass as bass
import concourse.bacc as bacc
import concourse.tile as tile
from concourse.bass_utils import run_bass_kernel_spmd

S = 128
B_FULL = 16384
N_CORES = 8
B = B_FULL // N_CORES  # 2048 per core
P = 128
NC = B // P  # 16

F32 = mybir.dt.float32
F16 = mybir.dt.float16
BF16 = mybir.dt.bfloat16
AF = mybir.ActivationFunctionType
ALU = mybir.AluOpType

NL = 4  # L ring slots (W=3 lookahead on the D/Ln pipeline)
W = 3


def build_kernel(a, c, s, tau, h, repeat=1):
    scale = 86400.0 * float(h)
    nc = bacc.Bacc()

    grt_in = nc.declare_dram_parameter("grt", [P, NC, S], BF16, isOutput=False)
    eye_in = nc.declare_dram_parameter("eye", [P, P], BF16, isOutput=False)
    out_ext = nc.declare_dram_parameter("out", [P, S, NC], F32, isOutput=True)

    with ExitStack() as ctx:
        tc = ctx.enter_context(tile.TileContext(nc))
        pool = ctx.enter_context(tc.tile_pool(name="p", bufs=1))

        GRT = pool.tile([P, NC, S], BF16)
        nc.sync.dma_start(out=GRT[:], in_=grt_in[:])
        EYE = pool.tile([P, P], BF16)
        nc.sync.dma_start(out=EYE[:], in_=eye_in[:])

        NEG = pool.tile([P, S, NC], F16)
        NEGP = pool.tile([P, S, NC], F32)

        LR = [pool.tile([P, S, NC], F16, name=f"LR{r}") for r in range(NL)]
        QB = [pool.tile([P, S, NC], F16, name=f"QB{r}") for r in range(2)]
        TB = [pool.tile([P, S, NC], F16, name=f"TB{r}") for r in range(2)]
        QF = [pool.tile([P, 1, NC], F16, name=f"QF{r}") for r in range(2)]
        TF = [pool.tile([P, 1, NC], F32, name=f"TF{r}") for r in range(2)]
        PR = [pool.tile([P, 1, NC], F32, name=f"PR{r}") for r in range(2)]
        BIAS = pool.tile([P, 1], F32)
        nc.vector.memset(BIAS[:], -float(tau) / float(s))

        psum_D = ctx.enter_context(nc.psum_tensor([P, S, NC], F32))

        def emit_inc(k):
            # psum_D[:, 0:k, :] += gap_k ; j-outer layout makes the prefix
            # contiguous: one matmul per touched 512-col psum bank.
            nj_total = k * NC
            j0 = 0
            while j0 * NC < nj_total:
                j1 = min(k, (j0 * NC + 512) // NC)
                pd = psum_D[:, j0:j1, :]
                outap = bass.AP(pd.tensor, pd.offset,
                                [pd.ap[0], [1, (j1 - j0) * NC]])
                g = GRT[:, 0, k : k + 1]
                mov = bass.AP(g.tensor, g.offset,
                              [g.ap[0], [0, j1 - j0], [S, NC]])
                nc.tensor.matmul(
                    outap, EYE[:], mov,
                    start=False, stop=False, skip_group_check=True,
                )
                j0 = j1

        def emit_ln(i, r):
            nc.scalar.activation(
                LR[r][:, 0:i, :], psum_D[:, 0:i, :], AF.Ln, scale=scale
            )

        for _rep in range(repeat):
            nc.vector.memset(NEG[:], -float(a))
            nc.vector.memset(NEGP[:], -float(a))
            nc.vector.memset(psum_D[:], 0.0)

            # prime the D/Ln pipeline W steps deep
            for k in range(1, min(W + 1, S)):
                emit_inc(k)
                emit_ln(k, k % NL)

            for i in range(1, S):
                r = i % NL
                x = i % 2
                # advance D for step i+W (waits Ln_{i+W-1}, which was
                # emitted last iteration — PE runs in ACT's shadow)
                if i + W < S:
                    emit_inc(i + W)
                # fresh chain for step i
                nc.vector.tensor_tensor(
                    out=QF[x][:], in0=LR[r][:, i - 1 : i, :],
                    in1=NEG[:, i - 1 : i, :], op=ALU.mult,
                )
                nc.scalar.activation(TF[x][:], QF[x][:], AF.Exp)
                # bulk for step i+1 over j < i
                if i + 1 < S:
                    r1 = (i + 1) % NL
                    x1 = (i + 1) % 2
                    nc.vector.tensor_tensor(
                        out=QB[x1][:, 0:i, :], in0=LR[r1][:, 0:i, :],
                        in1=NEG[:, 0:i, :], op=ALU.mult,
                    )
                    nc.scalar.activation(
                        TB[x1][:, 0:i, :], QB[x1][:, 0:i, :], AF.Exp
                    )
                if i + W < S:
                    emit_ln(i + W, (i + W) % NL)
                nc.vector.scalar_tensor_tensor(
                    out=NEG[:, i : i + 1, :], in0=TF[x][:], scalar=-float(c),
                    in1=NEGP[:, i : i + 1, :], op0=ALU.mult, op1=ALU.add,
                )
                if i + 1 < S:
                    tb = TB[x1][:, 0:i, :]
                    tbt = bass.AP(tb.tensor, tb.offset,
                                  [tb.ap[0], [1, NC], [NC, i]])
                    nc.vector.tensor_reduce(
                        out=PR[x1][:], in_=tbt,
                        axis=mybir.AxisListType.X, op=ALU.add,
                        opt_input=False,
                    )
                    nc.vector.tensor_scalar(
                        out=NEGP[:, i + 1 : i + 2, :], in0=PR[x1][:],
                        scalar1=-float(c), scalar2=-float(a),
                        op0=ALU.mult, op1=ALU.add,
                    )

        # epilogue
        SS = pool.tile([P, S, NC], F32)
        nc.vector.tensor_scalar(
            out=SS[:, 1:S, :], in0=NEG[:, 1:S, :],
            scalar1=float(a), scalar2=-1.0 / float(c),
            op0=ALU.add, op1=ALU.mult,
        )
        M = pool.tile([P, S, NC], F32)
        nc.scalar.activation(M[:, 1:S, :], SS[:, 1:S, :], AF.Ln)
        O = pool.tile([P, S, NC], F32)
        nc.vector.memset(O[:, 0:1, :], 0.0)
        nc.scalar.activation(
            O[:, 1:S, :], M[:, 1:S, :], AF.Sigmoid,
            scale=1.0 / float(s), bias=BIAS[:],
        )
        nc.sync.dma_start(out=out_ext[:], in_=O[:])

    nc.compile()
    return nc


def make_in_maps(sp: np.ndarray) -> list:
    bf = mybir.dt.np(BF16)
    eye = np.eye(P, dtype=np.float32).astype(bf)
    in_maps = []
    for ci in range(N_CORES):
        shard = sp[:, ci * B : (ci + 1) * B].astype(np.float32)  # [S, B]
        gaps = np.empty_like(shard)
        gaps[0] = shard[0]
        gaps[1:] = shard[1:] - shard[:-1]
        # grt[p, bc, j] = gaps[j, bc*128 + p]
        grt = np.ascontiguousarray(
            gaps.reshape(S, NC, P).transpose(2, 1, 0)
        ).astype(bf)
        in_maps.append({"grt": grt, "eye": eye})
    return in_maps


def kernel(sp: np.ndarray, w: np.ndarray) -> np.ndarray:
    sp = np.ascontiguousarray(sp, dtype=np.float32)
    w = np.asarray(w, dtype=np.float32)
    a, c, s, tau, h = (float(x) for x in w)

    nc = build_kernel(a, c, s, tau, h)
    in_maps = make_in_maps(sp)

    res = run_bass_kernel_spmd(nc, in_maps, core_ids=list(range(N_CORES)))
    outs = []
    for ci in range(N_CORES):
        o = res.results[ci]["out"]  # [P, S, NC]
        outs.append(o.transpose(1, 2, 0).reshape(S, B)[1:S])
    return np.concatenate(outs, axis=1).astype(np.float32)


if __name__ == "__main__":
    rng = np.random.default_rng(0)
    spt = np.cumsum(rng.uniform(0.1, 5.0, (S, B_FULL)).astype(np.float32), axis=0)
    wt = np.asarray(
        [0.176786766570677, 0.216967308403809, 0.254893976981164,
         -0.704205679427144, 0.025], dtype=np.float32)
    o = kernel(spt, wt)
    print(o.shape, o.dtype, o[:3, :3])


# revision 6
# speedup vs baseline: 1.8187x; 1.7525x over previous
"""ACT-R recurrence kernel, v2: transposed layout + PE-maintained diff state.

Layout: batch on partitions ([128 p] x [16 bc] x [128 j] free), per core
B=2048.  All TRN2 elementwise/reduce costs scale with free size only.

D state lives in PSUM [128, 16, 128] f32, maintained by the idle PE:
  step i: psum_D[:, :, 0:i] += gap_i  (identity-stationary matmul, bf16
  moving with a stride-0 broadcast AP; one accumulating matmul per bank).
After increment i, psum_D[:, :, j] = sp_i - sp_j for j < i.  Ln reads PSUM.

Per step (lag-1 split, S_i = P_i + F_i):
  Ln_i   (ACT):  L_i = Ln(scale * psum_D[0:i])            -> f16 ring
  QF_i   (DVE):  L_i[i-1] * NEG[i-1]          [128,16,1]
  ExpF_i (ACT):  F_i = Exp(QF_i)              [128,16,1]
  mul_i+1(DVE):  QB = L_{i+1}[0:i] * NEG[0:i]  f16 (2x mode)
  ExpB   (ACT):  TB = Exp(QB)                  f16
  red    (DVE):  P_{i+1} = reduce_add_X(TB)    f32
  stt_i  (DVE):  NEG[i] = -c*F_i + NEGP[i]     (fused)
  negp   (DVE):  NEGP[i+1] = -c*P_{i+1} - a
Final: S = (NEG + a)/(-c); out = Sigmoid((Ln(S) - tau)/s).
Ln and Exp share one activation table (get_activation_tables patch).
Single input: transposed bf16 gaps.  No gpsimd, no collectives.
"""

import sys

for _p in ("/opt/trn_rl_repo",):
    if _p not in sys.path:
        sys.path.insert(0, _p)

import numpy as np
from contextlib import ExitStack

import concourse.hw_specs as hw_specs
import concourse.bacc as bacc_mod
from concourse import mybir

_orig_gat = hw_specs.get_activation_tables


def _patched_gat(arch):
    tabs = _orig_gat(arch)
    out = {}
    ln_t = mybir.ActivationFunctionType.Ln
    exp_t = mybir.ActivationFunctionType.Exp
    for name, funcs in tabs.items():
        f = set(funcs)
        if name != "natural_log_exp_and_others":
            f.discard(ln_t)
            f.discard(exp_t)
        out[name] = f
    return out


bacc_mod.get_activation_tables = _patched_gat

import concourse.bass as bass
import concourse.bacc as bacc
import concourse.tile as tile
from concourse.bass_utils import run_bass_kernel_spmd

S = 128
B_FULL = 16384
N_CORES = 8
B = B_FULL // N_CORES  # 2048 per core
P = 128
NC = B // P  # 16

F32 = mybir.dt.float32
F16 = mybir.dt.float16
BF16 = mybir.dt.bfloat16
AF = mybir.ActivationFunctionType
ALU = mybir.AluOpType

NL = 5  # L ring slots (W=3 lookahead on the D/Ln pipeline)
W = 3


def build_kernel(a, c, s, tau, h, repeat=1):
    scale = 86400.0 * float(h)
    nc = bacc.Bacc()

    grt_in = nc.declare_dram_parameter("grt", [P, NC, S], BF16, isOutput=False)
    eye_in = nc.declare_dram_parameter("eye", [P, P], BF16, isOutput=False)
    out_ext = nc.declare_dram_parameter("out", [P, S, NC], F32, isOutput=True)

    with ExitStack() as ctx:
        tc = ctx.enter_context(tile.TileContext(nc))
        pool = ctx.enter_context(tc.tile_pool(name="p", bufs=1))

        GRT = pool.tile([P, NC, S], BF16)
        nc.sync.dma_start(out=GRT[:], in_=grt_in[:])
        EYE = pool.tile([P, P], BF16)
        nc.sync.dma_start(out=EYE[:], in_=eye_in[:])

        NEG = pool.tile([P, S, NC], F16)
        NEGP = pool.tile([P, S, NC], F32)

        LR = [pool.tile([P, S, NC], F16, name=f"LR{r}") for r in range(NL)]
        QB = [pool.tile([P, S, NC], F16, name=f"QB{r}") for r in range(3)]
        TB = [pool.tile([P, S, NC], F16, name=f"TB{r}") for r in range(3)]
        QF = [pool.tile([P, 1, NC], F16, name=f"QF{r}") for r in range(3)]
        TF = [pool.tile([P, 1, NC], F32, name=f"TF{r}") for r in range(3)]
        PR = [pool.tile([P, 1, NC], F32, name=f"PR{r}") for r in range(3)]
        BIAS = pool.tile([P, 1], F32)
        nc.vector.memset(BIAS[:], -float(tau) / float(s))

        psum_D = ctx.enter_context(nc.psum_tensor([P, S, NC], F32))

        def emit_inc(k):
            # psum_D[:, 0:k, :] += gap_k ; j-outer layout makes the prefix
            # contiguous: one matmul per touched 512-col psum bank.
            nj_total = k * NC
            j0 = 0
            while j0 * NC < nj_total:
                j1 = min(k, (j0 * NC + 512) // NC)
                pd = psum_D[:, j0:j1, :]
                outap = bass.AP(pd.tensor, pd.offset,
                                [pd.ap[0], [1, (j1 - j0) * NC]])
                g = GRT[:, 0, k : k + 1]
                mov = bass.AP(g.tensor, g.offset,
                              [g.ap[0], [0, j1 - j0], [S, NC]])
                nc.tensor.matmul(
                    outap, EYE[:], mov,
                    start=False, stop=False, skip_group_check=True,
                )
                j0 = j1

        def emit_ln(i, r):
            nc.scalar.activation(
                LR[r][:, 0:i, :], psum_D[:, 0:i, :], AF.Ln, scale=scale
            )

        for _rep in range(repeat):
            nc.vector.memset(NEG[:], -float(a))
            nc.vector.memset(NEGP[:], -float(a))
            nc.vector.memset(psum_D[:], 0.0)

            # prime the D/Ln pipeline W steps deep
            for k in range(1, min(W + 1, S)):
                emit_inc(k)
                emit_ln(k, k % NL)

            for i in range(1, S):
                r = i % NL
                x = i % 3
                # advance D for step i+W (waits Ln_{i+W-1}, which was
                # emitted last iteration — PE runs in ACT's shadow)
                if i + W < S:
                    emit_inc(i + W)
                # fresh chain for step i
                nc.vector.tensor_tensor(
                    out=QF[x][:], in0=LR[r][:, i - 1 : i, :],
                    in1=NEG[:, i - 1 : i, :], op=ALU.mult,
                )
                nc.scalar.activation(TF[x][:], QF[x][:], AF.Exp)
                # bulk for step i+1 over j < i
                if i + 1 < S:
                    r1 = (i + 1) % NL
                    x1 = (i + 1) % 3
                    nc.vector.tensor_tensor(
                        out=QB[x1][:, 0:i, :], in0=LR[r1][:, 0:i, :],
                        in1=NEG[:, 0:i, :], op=ALU.mult,
                    )
                    nc.scalar.activation(
                        TB[x1][:, 0:i, :], QB[x1][:, 0:i, :], AF.Exp
                    )
                if i + W < S:
                    emit_ln(i + W, (i + W) % NL)
                nc.vector.scalar_tensor_tensor(
                    out=NEG[:, i : i + 1, :], in0=TF[x][:], scalar=-float(c),
                    in1=NEGP[:, i : i + 1, :], op0=ALU.mult, op1=ALU.add,
                )
                if i + 1 < S:
                    tb = TB[x1][:, 0:i, :]
                    tbt = bass.AP(tb.tensor, tb.offset,
                                  [tb.ap[0], [1, NC], [NC, i]])
                    nc.vector.tensor_reduce(
                        out=PR[x1][:], in_=tbt,
                        axis=mybir.AxisListType.X, op=ALU.add,
                        opt_input=False,
                    )
                    nc.vector.tensor_scalar(
                        out=NEGP[:, i + 1 : i + 2, :], in0=PR[x1][:],
                        scalar1=-float(c), scalar2=-float(a),
                        op0=ALU.mult, op1=ALU.add,
                    )

        # epilogue
        SS = pool.tile([P, S, NC], F32)
        nc.vector.tensor_scalar(
            out=SS[:, 1:S, :], in0=NEG[:, 1:S, :],
            scalar1=float(a), scalar2=-1.0 / float(c),
            op0=ALU.add, op1=ALU.mult,
        )
        M = pool.tile([P, S, NC], F32)
        nc.scalar.activation(M[:, 1:S, :], SS[:, 1:S, :], AF.Ln)
        O = pool.tile([P, S, NC], F32)
        nc.vector.memset(O[:, 0:1, :], 0.0)
        nc.scalar.activation(
            O[:, 1:S, :], M[:, 1:S, :], AF.Sigmoid,
            scale=1.0 / float(s), bias=BIAS[:],
        )
        nc.sync.dma_start(out=out_ext[:], in_=O[:])

    nc.compile()
    return nc


def make_in_maps(sp: np.ndarray) -> list:
    bf = mybir.dt.np(BF16)
    eye = np.eye(P, dtype=np.float32).astype(bf)
    in_maps = []
    for ci in range(N_CORES):
        shard = sp[:, ci * B : (ci + 1) * B].astype(np.float32)  # [S, B]
        gaps = np.empty_like(shard)
        gaps[0] = shard[0]
        gaps[1:] = shard[1:] - shard[:-1]
        # grt[p, bc, j] = gaps[j, bc*128 + p]
        grt = np.ascontiguousarray(
            gaps.reshape(S, NC, P).transpose(2, 1, 0)
        ).astype(bf)
        in_maps.append({"grt": grt, "eye": eye})
    return in_maps


def kernel(sp: np.ndarray, w: np.ndarray) -> np.ndarray:
    sp = np.ascontiguousarray(sp, dtype=np.float32)
    w = np.asarray(w, dtype=np.float32)
    a, c, s, tau, h = (float(x) for x in w)

    nc = build_kernel(a, c, s, tau, h)
    in_maps = make_in_maps(sp)

    res = run_bass_kernel_spmd(nc, in_maps, core_ids=list(range(N_CORES)))
    outs = []
    for ci in range(N_CORES):
        o = res.results[ci]["out"]  # [P, S, NC]
        outs.append(o.transpose(1, 2, 0).reshape(S, B)[1:S])
    return np.concatenate(outs, axis=1).astype(np.float32)


if __name__ == "__main__":
    rng = np.random.default_rng(0)
    spt = np.cumsum(rng.uniform(0.1, 5.0, (S, B_FULL)).astype(np.float32), axis=0)
    wt = np.asarray(
        [0.176786766570677, 0.216967308403809, 0.254893976981164,
         -0.704205679427144, 0.025], dtype=np.float32)
    o = kernel(spt, wt)
    print(o.shape, o.dtype, o[:3, :3])
